# revision 12
# baseline (speedup 1.0000x reference)
"""Multi-head causal attention (B=4, T=2048, D=1024, H=16, Dh=64) on 8 trn2 cores.

Sharding: core c -> (batch b = c//2, head-group g = c%2 covering heads 8g..8g+7).
Each core computes QKV projection for its batch/head-group, causal attention,
and a partial output projection over its 512 head-dims.  Host sums the two
partial outputs per batch.

v2 schedule (per core, all matmul inputs bf16, fp32 PSUM accumulation):
  - attention ki-loop is software-pipelined: scores(ki+1) are issued before
    PV(ki), so the ScalarE exp(ki) hides under PE matmuls.
  - causal masking is ADDITIVE on the PSUM scores (DVE, -30000 on the strict
    lower triangle of S^T diag blocks) BEFORE exp, off the exp->PV chain.
  - a "filler" generator interleaves projection / out-projection matmuls into
    every attention step so the PE never starves while ACT runs exp.
  - PSUM budget: 2x score tiles [128,1024] (4 banks) + 1 PV pair [65,1024]
    (2 banks) + 1 proj/outproj accumulator [128,1024] (2 banks) = 8 banks.
  - evacuations balanced: DVE does qk/v/stage copies + normalize muls,
    ScalarE does half the outproj evacuations, gpsimd does the partition
    broadcast of the softmax reciprocal.
"""

import sys

if "/opt/trn_rl_repo" not in sys.path:
    sys.path.insert(0, "/opt/trn_rl_repo")

from contextlib import ExitStack

import ml_dtypes
import numpy as np

import concourse.bass as bass
import concourse.bacc as bacc
import concourse.mybir as mybir
import concourse.tile as tile
from concourse.bass_utils import run_bass_kernel_spmd

BF16 = mybir.dt.bfloat16
F32 = mybir.dt.float32
NPBF16 = ml_dtypes.bfloat16

B, T, D = 4, 2048, 1024
H, DH = 16, 64
HPG = 8          # heads per group (per core)
GD = HPG * DH    # 512 head-dims per core
NT = T // 128    # 16 t-blocks
NC = D // 128    # 8 model-dim chunks
NQ = T // 512    # 4 q-spans
SCALE = 1.0 / 8.0
NEG = -30000.0   # additive mask value; exp(NEG) == 0


class Filler:
    """Queue of generators emitting PE work in small chunks (between yields)."""

    def __init__(self):
        self.queue = []
        self.cur = None
        self.tok = None
        self.done = set()

    def add(self, token, gen):
        self.queue.append((token, gen))

    def step(self):
        while True:
            if self.cur is None:
                if not self.queue:
                    return False
                self.tok, self.cur = self.queue.pop(0)
            try:
                next(self.cur)
                return True
            except StopIteration:
                self.done.add(self.tok)
                self.cur = None

    def require(self, token):
        while token not in self.done:
            if not self.step() and token not in self.done:
                raise RuntimeError(f"filler exhausted before {token}")

    def drain_all(self):
        while self.step():
            pass


def build_attention_kernel(mode: str = "causal", reps: int = 1, phases: str = "all",
                           unroll: bool = False):
    """mode: 'causal' (tril mask), 'dense' (all-ones mask), 'masked' (arbitrary).

    reps > 1 wraps the compute body in a hardware For_i loop (for timing the
    kernel body without host dispatch overhead); unroll=True python-unrolls
    instead (for TimelineSim, which can't resolve For_i reg branches).
    phases: 'all' | 'proj'."""
    nc = bacc.Bacc("TRN2", target_bir_lowering=False)

    xT_d = nc.dram_tensor("xT", [D, T], BF16, kind="ExternalInput")
    wq_d = nc.dram_tensor("wq", [D, GD], BF16, kind="ExternalInput")
    wk_d = nc.dram_tensor("wk", [D, GD], BF16, kind="ExternalInput")
    wv_d = nc.dram_tensor("wv", [D, GD], BF16, kind="ExternalInput")
    wo_d = nc.dram_tensor("wout", [GD, D], BF16, kind="ExternalInput")
    amask_d = nc.dram_tensor("amask", [128, 128], BF16, kind="ExternalInput")
    if mode == "masked":
        m01_d = nc.dram_tensor("m01T", [T, T], BF16, kind="ExternalInput")
    out_d = nc.dram_tensor("out", [T, D], F32, kind="ExternalOutput")

    with tile.TileContext(nc) as tc, ExitStack() as ctx:
        const = ctx.enter_context(tc.tile_pool(name="const", bufs=1))
        ppool = ctx.enter_context(tc.tile_pool(name="ppool", bufs=4))
        rpool = ctx.enter_context(tc.tile_pool(name="rpool", bufs=4))
        bpool = ctx.enter_context(tc.tile_pool(name="bpool", bufs=4))
        opool = ctx.enter_context(tc.tile_pool(name="opool", bufs=4))
        mpool = ctx.enter_context(tc.tile_pool(name="mpool", bufs=4))
        spool = ctx.enter_context(
            tc.tile_pool(name="spool", bufs=2, space=bass.MemorySpace.PSUM)
        )
        pvpool = ctx.enter_context(
            tc.tile_pool(name="pvpool", bufs=1, space=bass.MemorySpace.PSUM)
        )
        hpool = ctx.enter_context(
            tc.tile_pool(name="hpool", bufs=2, space=bass.MemorySpace.PSUM)
        )

        # ---- load inputs --------------------------------------------------
        xT = const.tile([128, NC, T], BF16)
        wq = const.tile([128, NC, GD], BF16)
        wk = const.tile([128, NC, GD], BF16)
        wv = const.tile([128, NC, GD], BF16)
        for c in range(NC):
            nc.sync.dma_start(xT[:, c, :], xT_d[c * 128 : (c + 1) * 128, :])
            nc.sync.dma_start(wq[:, c, :], wq_d[c * 128 : (c + 1) * 128, :])
            nc.sync.dma_start(wk[:, c, :], wk_d[c * 128 : (c + 1) * 128, :])
            nc.sync.dma_start(wv[:, c, :], wv_d[c * 128 : (c + 1) * 128, :])
        wo = const.tile([128, GD // 128, D], BF16)
        for c in range(GD // 128):
            nc.sync.dma_start(wo[:, c, :], wo_d[c * 128 : (c + 1) * 128, :])
        amask = const.tile([128, 128], BF16)
        nc.sync.dma_start(amask[:], amask_d[:])

        qT = const.tile([128, GD // 128, T], BF16)
        kT = const.tile([128, GD // 128, T], BF16)
        v = const.tile([128, NT, HPG, DH + 1], BF16)
        ot = const.tile([128, GD // 128, T], BF16)

        nc.vector.memset(v[:, :, :, DH : DH + 1], 1.0)

        def body():
            _body(nc, tc, mode, spool, pvpool, hpool, ppool, rpool, bpool,
                  opool, mpool, xT, wq, wk, wv, wo, amask, qT, kT, v, ot,
                  m01_d if mode == "masked" else None, out_d, phases)

        if unroll:
            for _ in range(reps):
                body()
        elif reps > 1:
            with tc.For_i(0, reps, 1):
                body()
        else:
            body()

    nc.compile()
    return nc


def _body(nc, tc, mode, spool, pvpool, hpool, ppool, rpool, bpool, opool,
          mpool, xT, wq, wk, wv, wo, amask, qT, kT, v, ot, m01_d, out_d,
          phases="all"):
    filler = Filler()

    # ---- filler units (each yield ~= 2 matmuls of N=512) -----------------
    # span-major over two single-bank accumulator slots: half A's evacuation
    # (DVE/ACT copy) hides under half B's matmuls, so the PE FIFO never
    # blocks on a PSUM slot.
    def qk_unit(w, dst, j, np2):
        for ni in range(2):
            n = 2 * np2 + ni
            acc = hpool.tile([128, 512], F32, tag="acc", name="accqk")
            for c in range(NC):
                nc.tensor.matmul(
                    acc[:],
                    w[:, c, j * 128 : (j + 1) * 128],
                    xT[:, c, n * 512 : (n + 1) * 512],
                    start=(c == 0),
                    stop=(c == NC - 1),
                )
                if c % 2 == 1:
                    yield
            nc.vector.tensor_copy(dst[:, j, n * 512 : (n + 1) * 512], acc[:])

    def v_unit(t0):
        for ti in range(2):
            t = t0 + ti
            acc = hpool.tile([128, 512], F32, tag="acc", name="accv")
            for c in range(NC):
                nc.tensor.matmul(
                    acc[:],
                    xT[:, c, t * 128 : (t + 1) * 128],
                    wv[:, c, :],
                    start=(c == 0),
                    stop=(c == NC - 1),
                )
                if c % 2 == 1:
                    yield
            nc.vector.tensor_copy(
                v[:, t, :, 0:DH], acc[:].rearrange("p (h e) -> p h e", e=DH)
            )

    def op_unit(t):
        for n2 in range(2):
            acc = hpool.tile([128, 512], F32, tag="acc", name="acco")
            for c in range(GD // 128):
                nc.tensor.matmul(
                    acc[:],
                    ot[:, c, t * 128 : (t + 1) * 128],
                    wo[:, c, n2 * 512 : (n2 + 1) * 512],
                    start=(c == 0),
                    stop=(c == GD // 128 - 1),
                )
                if c % 2 == 1:
                    yield
            ob = opool.tile([128, 512], F32, tag="ob")
            if (t + n2) % 2 == 0:
                nc.vector.tensor_copy(ob[:], acc[:])
            else:
                nc.scalar.copy(ob[:], acc[:])
            nc.sync.dma_start(
                out_d[t * 128 : (t + 1) * 128, n2 * 512 : (n2 + 1) * 512], ob[:]
            )

    # stream order chosen so att(qi, j) dependencies appear before their
    # forced-drain point (see require() calls below)
    for j in range(4):
        filler.add(("q", j, 0), qk_unit(wq, qT, j, 0))
        filler.add(("k", j, 0), qk_unit(wk, kT, j, 0))
        if j == 0:
            filler.add(("v", 0), v_unit(0))
            filler.add(("v", 2), v_unit(2))
        if j == 1:
            filler.add(("v", 4), v_unit(4))
            filler.add(("v", 6), v_unit(6))

    def add_np2_1():
        for j in range(4):
            filler.add(("q", j, 1), qk_unit(wq, qT, j, 1))
            filler.add(("k", j, 1), qk_unit(wk, kT, j, 1))
            if j == 1:
                filler.add(("v", 8), v_unit(8))
                filler.add(("v", 10), v_unit(10))
        # v for t 12..15 go last: they are only needed late in att(3, *) and
        # serve as in-attention filler there (required just-in-time below)
        filler.add(("v", 12), v_unit(12))
        filler.add(("v", 14), v_unit(14))

    # ---- attention --------------------------------------------------------
    amask2 = bass.AP(
        tensor=amask.tensor,
        offset=amask.offset,
        ap=[list(amask.ap[0]), [0, 2], list(amask.ap[1])],
    )

    def att(qi, j):
        q0 = qi * 512
        nki = 4 * qi + 4 if mode == "causal" else NT
        pv = pvpool.tile([65, 1024], F32, tag="pv")

        def emit_pv(ki, p2, lo):
            for hh in range(2):
                nc.tensor.matmul(
                    pv[0:65, hh * 512 + lo : (hh + 1) * 512],
                    v[:, ki, 2 * j + hh, :],
                    p2[:, hh * 512 + lo : (hh + 1) * 512],
                    start=(ki == 0),
                    stop=(ki == nki - 1),
                )

        prev = None
        for ki in range(nki):
            # just-in-time: the v-projection covering block ki must be in
            # the emitted stream before PV(ki) (next iteration)
            filler.require(("v", ki - ki % 2))
            d = ki - 4 * qi  # >=0: diagonal band (causal mode only)
            lo = max(d, 0) * 128 if mode == "causal" else 0
            if mode == "masked":
                m01 = mpool.tile([128, 512], BF16, tag="m01")
                nc.sync.dma_start(
                    m01[:], m01_d[ki * 128 : (ki + 1) * 128, q0 : q0 + 512]
                )
            s2 = spool.tile([128, 1024], F32, tag="s2")
            for hh in range(2):
                nc.tensor.matmul(
                    s2[:, hh * 512 + lo : (hh + 1) * 512],
                    kT[hh * 64 : (hh + 1) * 64, j, ki * 128 : (ki + 1) * 128],
                    qT[hh * 64 : (hh + 1) * 64, j, q0 + lo : q0 + 512],
                    start=True,
                    stop=True,
                )
            if prev is not None:
                emit_pv(*prev)
            if mode == "causal" and d >= 0:
                # additive causal mask on the 128-wide diagonal sub-block,
                # applied to PSUM scores before exp (hides under exp(ki-1))
                sv = s2.rearrange("p (h w) -> p h w", h=2)[:, :, lo : lo + 128]
                nc.vector.tensor_add(sv, sv, amask2)
            p2 = ppool.tile([128, 1024], BF16, tag="p2")
            if lo == 0:
                nc.scalar.activation(p2[:], s2[:], mybir.ActivationFunctionType.Exp)
            else:
                sv2 = s2.rearrange("p (h w) -> p h w", h=2)[:, :, lo:512]
                pw = p2.rearrange("p (h w) -> p h w", h=2)[:, :, lo:512]
                nc.scalar.activation(pw, sv2, mybir.ActivationFunctionType.Exp)
            if mode == "masked":
                pm = p2.rearrange("p (h w) -> p h w", h=2)
                m2 = bass.AP(
                    tensor=m01.tensor,
                    offset=m01.offset,
                    ap=[list(m01.ap[0]), [0, 2], list(m01.ap[1])],
                )
                nc.vector.tensor_mul(pm, pm, m2)
            prev = (ki, p2, lo)
            filler.step()
        emit_pv(*prev)

        # ---- normalize: stage -> recip -> broadcast -> muls ---------------
        # per-head stage copies on separate engines release the pv banks
        # quickly so the next (qi, j) block's PV matmuls don't stall
        stage = rpool.tile([65, 1024], F32, tag="stage")
        nc.vector.tensor_copy(stage[:, 0:512], pv[:, 0:512])
        nc.scalar.copy(stage[:, 512:1024], pv[:, 512:1024])
        recip = rpool.tile([1, 1024], F32, tag="recip")
        nc.vector.reciprocal(recip[:], stage[64:65, :])
        bc = bpool.tile([64, 1024], F32, tag="bc")
        nc.gpsimd.partition_broadcast(bc[:], recip[:])
        nc.vector.tensor_mul(
            ot[0:64, j, q0 : q0 + 512], stage[0:64, 0:512], bc[:, 0:512]
        )
        otn = opool.tile([64, 512], BF16, tag="otn")
        nc.vector.tensor_mul(otn[:], stage[0:64, 512:1024], bc[:, 512:1024])
        nc.sync.dma_start(ot[64:128, j, q0 : q0 + 512], otn[:])

    # ---- schedule ---------------------------------------------------------
    if phases == "proj":
        add_np2_1()
        filler.drain_all()
        return

    for qi in range(NQ):
        if qi == 2:
            add_np2_1()
        for j in range(4):
            np2s = [0] if qi < 2 else [0, 1]
            for np2 in np2s:
                filler.require(("q", j, np2))
                filler.require(("k", j, np2))
            att(qi, j)
        for t in range(4 * qi, 4 * qi + 4):
            filler.add(("op", t), op_unit(t))
    filler.drain_all()


_NC_CACHE: dict = {}


def _get_kernel(mode: str, reps: int = 1, phases: str = "all"):
    key = (mode, reps, phases)
    if key not in _NC_CACHE:
        _NC_CACHE[key] = build_attention_kernel(mode, reps, phases)
    return _NC_CACHE[key]


def make_in_maps(x, mask, Wqkv, Wout):
    tril = np.tril(np.ones((T, T), dtype=np.int32))
    m = np.asarray(mask[0, 0])
    if np.array_equal(m, tril):
        mode = "causal"
    elif np.all(m == 1):
        mode = "dense"
    else:
        mode = "masked"

    ii, jj = np.indices((128, 128))
    amask128 = np.where(ii > jj, NEG, 0.0).astype(NPBF16)
    in_maps = []
    for c in range(8):
        b, g = c // 2, c % 2
        im = {
            "xT": np.ascontiguousarray(x[b].T).astype(NPBF16),
            "wq": Wqkv[:, g * GD : (g + 1) * GD].astype(NPBF16),
            "wk": (Wqkv[:, D + g * GD : D + (g + 1) * GD] * SCALE).astype(NPBF16),
            "wv": Wqkv[:, 2 * D + g * GD : 2 * D + (g + 1) * GD].astype(NPBF16),
            "wout": Wout[g * GD : (g + 1) * GD, :].astype(NPBF16),
            "amask": amask128,
        }
        if mode == "masked":
            im["m01T"] = np.ascontiguousarray(m.T).astype(NPBF16)
        in_maps.append(im)
    return mode, in_maps


def kernel(x, mask, Wqkv, Wout):
    x = np.asarray(x)
    mask = np.asarray(mask)
    Wqkv = np.asarray(Wqkv)
    Wout = np.asarray(Wout)
    mode, in_maps = make_in_maps(x, mask, Wqkv, Wout)
    nc = _get_kernel(mode)
    res = run_bass_kernel_spmd(nc, in_maps, core_ids=list(range(8)))
    out = np.zeros((B, T, D), dtype=np.float32)
    for c in range(8):
        out[c // 2] += res.results[c]["out"]
    return out
